# revision 35
# baseline (speedup 1.0000x reference)
"""Trainium2 Bass kernel for nn_AstPathEncoder (bidirectional LSTM + segment-mean).

Strategy (8 NeuronCores, data-parallel over paths):
  - Each core takes 2048 paths = 16 whole samples; weights replicated.
  - Per core, paths are sorted by descending length; at LSTM step t only the
    first W[t] (live, padded to 128) paths are processed.  W[t] is the max
    over cores so all 8 cores share one SPMD graph; per-core variation flows
    through input arrays only (gather indices, pad flags, the segment
    indicator matrix).
  - Embedding rows are fetched with dma_gather(transpose=True), which lands
    x^T [E x tokens] tiles directly in SBUF (gather + transpose fused).
  - Gates are computed transposed: gates[4H, n] = W_ih @ x^T + W_hh @ h^T
    (+bias via ACT), accumulated in PSUM with bf16 matmuls.
  - Backward dir: padding tokens force the input gate to -BIG via a K=1
    matmul with a per-core pad-flag row, so dead lanes keep (h,c) = 0 with
    no masking (packed-sequence semantics for the reverse scan).
  - Forward dir: final h per path is captured at its death step with a
    predicated copy against a broadcast death mask; the mask is built on the
    fly as (pad[t+1] - pad[t]) via two K=1 matmuls (padding is monotone).
  - Pooling: h_n^T is PE-transposed, then one indicator matmul produces the
    per-sample means (indicator holds 1/len); linear layer + L2 norm on-chip.
"""

import numpy as np
import ml_dtypes

import concourse.bass as bass
import concourse.mybir as mybir
import concourse.tile as tile
from concourse import bacc
from concourse.bass_utils import run_bass_kernel_spmd

F32 = mybir.dt.float32
BF16 = mybir.dt.bfloat16
FP8 = mybir.dt.float8e4
I16 = mybir.dt.int16
AF = mybir.ActivationFunctionType
OP = mybir.AluOpType

N, T, V, E, H, D, B = 16384, 16, 10000, 256, 256, 512, 128
NCORES = 8
BIG = 30.0
SCALE_S = 8.0   # fp8 x-path scale: emb*4, W_ih*2, W_hh*8, ACT scale 1/8

_NC_CACHE = {}

CDT = BF16  # cell-state dtype knob (BF16 or F32)

FP8_NP = ml_dtypes.float8_e4m3fn


def _flag_offsets(sched_g, nloc):
    """Stored width for pad row t is Wg[t-1] (Wg[-1] := nloc)."""
    sw = [nloc] + list(sched_g[:-1])
    poff = np.concatenate([[0], np.cumsum(sw)]).astype(int)
    return sw, poff


def build_nc(sched, sched_g, nloc, spc, min_s, max_s):
    nblk = nloc // 128
    gcols_per = [w // 16 for w in sched_g]
    goff = np.concatenate([[0], np.cumsum(gcols_per)]).astype(int)
    gcols = int(goff[-1])
    _, poff = _flag_offsets(sched_g, nloc)
    flag_len = int(poff[-1])

    nc = bacc.Bacc("TRN2", num_devices=NCORES)

    emb_d = nc.dram_tensor("emb", [V, E], FP8, kind="ExternalInput")
    gidx_d = nc.dram_tensor("gidx", [128, gcols], I16, kind="ExternalInput")
    pad_d = nc.dram_tensor("padflag", [1, flag_len], FP8, kind="ExternalInput")
    ind_d = nc.dram_tensor("ind", [128, nblk * spc], BF16, kind="ExternalInput")
    wihf_d = nc.dram_tensor("wihf", [128, 2 * 1024], FP8, kind="ExternalInput")
    whhf_d = nc.dram_tensor("whhf", [128, 2 * 1024], BF16, kind="ExternalInput")
    wihb_d = nc.dram_tensor("wihb", [128, 2 * 1024], FP8, kind="ExternalInput")
    whhb_d = nc.dram_tensor("whhb", [128, 2 * 1024], BF16, kind="ExternalInput")
    wlt_d = nc.dram_tensor("wlt", [128, 4 * D], F32, kind="ExternalInput")
    b4f_d = nc.dram_tensor("b4f", [128, 8], F32, kind="ExternalInput")
    b4b_d = nc.dram_tensor("b4b", [128, 8], F32, kind="ExternalInput")
    blin_d = nc.dram_tensor("blin", [128, 4], F32, kind="ExternalInput")
    out_d = nc.dram_tensor("out", [spc, D], F32, kind="ExternalOutput")

    with tile.TileContext(nc) as tc:
        with tc.tile_pool(name="persist", bufs=1) as pp:
            wihf = pp.tile([128, 2, 1024], FP8, tag="wihf")
            whhf = pp.tile([128, 2 * 1024], BF16, tag="whhf")
            wihb = pp.tile([128, 2, 1024], FP8, tag="wihb")
            whhb = pp.tile([128, 2 * 1024], BF16, tag="whhb")
            wlt = pp.tile([128, 4 * D], F32, tag="wlt")
            b4f = pp.tile([128, 8], F32, tag="b4f")
            b4b = pp.tile([128, 8], F32, tag="b4b")
            blin = pp.tile([128, 4], F32, tag="blin")
            gidx = pp.tile([128, gcols], I16, tag="gidx")
            padf = pp.tile([1, flag_len], FP8, tag="padf")
            ones_row = pp.tile([1, nloc], FP8, tag="ones_row")
            ind = pp.tile([128, nblk * spc], BF16, tag="ind")
            ones_c = pp.tile([1, 128], FP8, tag="ones_c")
            negones_c = pp.tile([1, 128], FP8, tag="negones_c")
            bigneg = pp.tile([1, 128], FP8, tag="bigneg")
            ident = pp.tile([128, 128], BF16, tag="ident")
            ident32 = pp.tile([128, 128], F32, tag="ident32")
            ident8 = pp.tile([128, 128], FP8, tag="ident8")
            # state (single-buffered h per dir: WAR deps follow the natural
            # per-step order, so no ping-pong needed)
            h_f = pp.tile([128, 2, nloc], BF16, tag="h_f")
            h_b = pp.tile([128, 2, nloc], BF16, tag="h_b")
            c_f = pp.tile([128, 2, nloc], CDT, tag="c_f")
            c_b = pp.tile([128, 2, nloc], CDT, tag="c_b")
            hf_fin = pp.tile([128, 2, nloc], BF16, tag="hffin")

            # gidx goes through the gpsimd queue so the first gather
            # follows it in-order with no cross-engine DMA-batch wait
            nc.gpsimd.dma_start(out=gidx[:], in_=gidx_d[:])
            nc.sync.dma_start(
                out=wihf[:], in_=wihf_d[:].rearrange("p (j m) -> p j m", j=2))
            nc.sync.dma_start(
                out=wihb[:], in_=wihb_d[:].rearrange("p (j m) -> p j m", j=2))
            for dst, src in ((padf, pad_d),
                             (whhf, whhf_d), (whhb, whhb_d),
                             (b4f, b4f_d), (b4b, b4b_d)):
                nc.sync.dma_start(out=dst[:], in_=src[:])
            nc.gpsimd.memset(ones_c[:], 1.0)
            nc.gpsimd.memset(negones_c[:], -1.0)
            nc.gpsimd.memset(bigneg[:], -BIG * SCALE_S)
            nc.gpsimd.memset(ones_row[:], 1.0)
            from concourse.masks import make_identity
            make_identity(nc, ident[:])
            make_identity(nc, ident32[:])
            make_identity(nc, ident8[:])

            def w_slice(wt, k, m):
                return wt[:, k * 1024 + m * 128:k * 1024 + (m + 1) * 128]

            def pad_sl(t, a, b):
                return padf[0:1, int(poff[t]) + a:int(poff[t]) + b]

            with tc.tile_pool(name="xt", bufs=4) as xp, \
                 tc.tile_pool(name="gates", bufs=2) as gp, \
                 tc.tile_pool(name="dvet", bufs=2) as vp, \
                 tc.tile_pool(name="psum", bufs=4, space="PSUM") as psp:

                xtiles = {}

                XCACHE_T0 = 8

                def emit_gather(dkey, t):
                    w = sched_g[t]
                    if w == 0:
                        return
                    if dkey == "b" and t >= XCACHE_T0:
                        xt = xp.tile([128, 2 * w], FP8, tag=f"xc{t}",
                                     name=f"xc{t}", bufs=1)
                    else:
                        xt = xp.tile([128, 2 * w], FP8, tag="xt", name="xt")
                    nc.gpsimd.dma_gather(
                        out_ap=xt[:].rearrange("p (c w) -> p c w", c=2),
                        in_ap=emb_d[:],
                        idxs_ap=gidx[:, int(goff[t]):int(goff[t + 1])],
                        num_idxs=w,
                        num_idxs_reg=w,
                        elem_size=E,
                        transpose=True,
                        single_packet=False,
                    )
                    xtiles[(dkey, t)] = xt
                    if dkey == "b" and t >= XCACHE_T0:
                        xtiles[("f", t)] = xt

                def emit_step(t, fwd):
                    w = sched[t]
                    if w == 0:
                        return
                    first = (t == 0) if fwd else (t == T - 1)
                    wih = wihf if fwd else wihb
                    whh = whhf if fwd else whhb
                    b4 = b4f if fwd else b4b
                    h_st = h_f if fwd else h_b
                    cbuf = c_f if fwd else c_b
                    xt = xtiles.pop(("f" if fwd else "b", t))
                    gmap = ("i", "i", "f", "f", "g", "g", "o", "o")
                    gt = {}
                    for gname in ("i", "f", "g", "o"):
                        if first and gname == "f":
                            continue
                        gt[gname] = gp.tile([128, 2, w], BF16,
                                            tag="g" + gname, name="g" + gname)
                    death_lo = min_s[t + 1] if t < T - 1 else 0
                    death_hi = max_s[t]
                    has_death = fwd and (w > death_lo) and (death_hi > 0)
                    mk8 = None
                    if has_death:
                        mk8 = vp.tile([128, w], mybir.dt.uint8,
                                      tag="mk8", name="mk8", bufs=1)
                    for w0 in range(0, w, 1024):
                        wh = min(1024, w - w0)
                        # sigmoids (i, f, o) first, tanh (g) last per half
                        for m in (0, 1, 2, 3, 6, 7, 4, 5):
                            gname = gmap[m]
                            if first and gname == "f":
                                continue
                            ps = psp.tile([128, 1024], F32, tag="gp", name="gp")
                            n_k = 1 if first else 3
                            extra = (1 if (not fwd and m < 2
                                           and w0 + wh > min_s[t]) else 0)

                            def sub_extra(ws, sw):
                                return (extra and w0 + ws + sw > min_s[t])
                            ki = 0
                            for ws in range(0, wh, 512):
                                sw = min(512, wh - ws)
                                n_tot = n_k + (1 if sub_extra(ws, sw) else 0)
                                a = w0 + ws
                                rhs8 = xt[:, 2 * a:2 * (a + sw)].rearrange(
                                    "p (w j) -> p j w", j=2)
                                nc.tensor.matmul(
                                    out=ps[:, ws:ws + sw],
                                    lhsT=wih[:, :, m * 128:(m + 1) * 128],
                                    rhs=rhs8,
                                    start=True,
                                    stop=(ki == n_tot - 1),
                                    perf_mode=mybir.MatmulPerfMode.DoubleRow,
                                )
                            ki += 1
                            if not first:
                                for k in range(2):
                                    for ws in range(0, wh, 512):
                                        sw = min(512, wh - ws)
                                        n_tot = n_k + (1 if sub_extra(ws, sw) else 0)
                                        nc.tensor.matmul(
                                            out=ps[:, ws:ws + sw],
                                            lhsT=w_slice(whh, k, m),
                                            rhs=h_st[:, k, w0 + ws:w0 + ws + sw],
                                            start=False,
                                            stop=(ki == n_tot - 1),
                                        )
                                    ki += 1
                            if extra:
                                for ws in range(0, wh, 512):
                                    sw = min(512, wh - ws)
                                    if not sub_extra(ws, sw):
                                        continue
                                    nc.tensor.matmul(
                                        out=ps[:, ws:ws + sw],
                                        lhsT=bigneg[:],
                                        rhs=pad_sl(t, w0 + ws, w0 + ws + sw),
                                        start=False,
                                        stop=True,
                                    )
                            func = AF.Tanh if gname == "g" else AF.Sigmoid
                            nc.scalar.activation(
                                out=gt[gname][:, m % 2, w0:w0 + wh],
                                in_=ps[:, :wh],
                                func=func,
                                bias=b4[:, m:m + 1],
                                scale=1.0 / SCALE_S,
                            )
                        if has_death and (w0 + wh > death_lo):
                            mask_ps = psp.tile([128, 1024], F32, tag="gp",
                                               name="mask_ps")
                            for ws in range(0, wh, 512):
                                sw = min(512, wh - ws)
                                a, bnd = w0 + ws, w0 + ws + sw
                                rhs_next = (pad_sl(t + 1, a, bnd) if t < T - 1
                                            else ones_row[0:1, a:bnd])
                                nc.tensor.matmul(
                                    out=mask_ps[:, ws:ws + sw], lhsT=ones_c[:],
                                    rhs=rhs_next, start=True, stop=False)
                                nc.tensor.matmul(
                                    out=mask_ps[:, ws:ws + sw], lhsT=negones_c[:],
                                    rhs=pad_sl(t, a, bnd),
                                    start=False, stop=True)
                            nc.vector.tensor_copy(mk8[:, w0:w0 + wh],
                                                  mask_ps[:, :wh])
                        # cell update on DVE for this half
                        c_sl = cbuf[:, :, w0:w0 + wh]
                        if first:
                            nc.vector.tensor_tensor(
                                out=c_sl, in0=gt["i"][:, :, w0:w0 + wh],
                                in1=gt["g"][:, :, w0:w0 + wh], op=OP.mult)
                        else:
                            ig = vp.tile([128, 2, wh], BF16, tag="ig",
                                         name="ig", bufs=1)
                            nc.vector.tensor_tensor(
                                out=ig[:], in0=gt["i"][:, :, w0:w0 + wh],
                                in1=gt["g"][:, :, w0:w0 + wh], op=OP.mult)
                            nc.vector.tensor_tensor(
                                out=c_sl, in0=gt["f"][:, :, w0:w0 + wh],
                                in1=c_sl, op=OP.mult)
                            nc.vector.tensor_tensor(
                                out=c_sl, in0=c_sl, in1=ig[:], op=OP.add)
                        tc_t = vp.tile([128, 2, wh], BF16, tag="tc", name="tc")
                        nc.scalar.activation(out=tc_t[:], in_=c_sl, func=AF.Tanh)
                        h_sl = h_st[:, :, w0:w0 + wh]
                        nc.vector.tensor_tensor(
                            out=h_sl, in0=gt["o"][:, :, w0:w0 + wh],
                            in1=tc_t[:], op=OP.mult)
                        if mk8 is not None and (w0 + wh > death_lo):
                            for ch in range(2):
                                nc.vector.copy_predicated(
                                    out=hf_fin[:, ch, w0:w0 + wh],
                                    mask=mk8[:, w0:w0 + wh],
                                    data=h_st[:, ch, w0:w0 + wh],
                                )

                emit_gather("b", T - 1)
                emit_gather("f", 0)
                nc.vector.memset(h_b[:], 0.0)
                nc.vector.memset(c_b[:], 0.0)
                for i in range(T):
                    if i + 1 < T:
                        if i + 1 < XCACHE_T0:
                            emit_gather("f", i + 1)
                        emit_gather("b", T - 2 - i)
                    emit_step(i, fwd=True)
                    emit_step(T - 1 - i, fwd=False)

            for dst, src in ((wlt, wlt_d), (blin, blin_d), (ind, ind_d)):
                nc.sync.dma_start(out=dst[:], in_=src[:])
            # ---------------- tail: pooling + linear + L2 norm ----------------
            with tc.tile_pool(name="tailsb", bufs=2) as tsb, \
                 tc.tile_pool(name="tailps", bufs=1, space="PSUM") as tps, \
                 tc.tile_pool(name="tailps2", bufs=2, space="PSUM") as tps2:
                pool_ps = tps.tile([spc, D], F32, tag="pool16")
                for j in range(nblk):
                    tp = tps2.tile([128, 512], BF16, tag="tp", name="tp")
                    for q in range(4):
                        src = (hf_fin if q < 2 else h_b)
                        ch = q % 2
                        nc.tensor.transpose(
                            out=tp[:, q * 128:(q + 1) * 128],
                            in_=src[:, ch, j * 128:(j + 1) * 128],
                            identity=ident[:],
                        )
                    hnt = tsb.tile([128, 512], BF16, tag="hnt", name="hnt")
                    nc.scalar.copy(hnt[:], tp[:])
                    nc.tensor.matmul(
                        out=pool_ps[:],
                        lhsT=ind[:, j * spc:(j + 1) * spc],
                        rhs=hnt[:],
                        start=(j == 0),
                        stop=(j == nblk - 1),
                    )
                pool_sb = tsb.tile([spc, D], F32, tag="poolsb")
                nc.scalar.copy(pool_sb[:], pool_ps[:])
                pt_ps = tps.tile([128, 4 * spc], F32, tag="ptps")
                for q in range(4):
                    nc.tensor.transpose(
                        out=pt_ps[:, q * spc:(q + 1) * spc],
                        in_=pool_sb[:, q * 128:(q + 1) * 128],
                        identity=ident32[:spc, :spc],
                    )
                pt_sb = tsb.tile([128, 4 * spc], F32, tag="ptsb")
                nc.scalar.copy(pt_sb[:], pt_ps[:])
                rt_ps = tps.tile([128, 4 * spc], F32, tag="rtps")
                for m in range(4):
                    for k in range(4):
                        nc.tensor.matmul(
                            out=rt_ps[:, m * spc:(m + 1) * spc],
                            lhsT=wlt[:, k * D + m * 128:k * D + (m + 1) * 128],
                            rhs=pt_sb[:, k * spc:(k + 1) * spc],
                            start=(k == 0),
                            stop=(k == 3),
                        )
                rt_sb = tsb.tile([128, 4 * spc], F32, tag="rtsb")
                for m in range(4):
                    nc.scalar.activation(
                        out=rt_sb[:, m * spc:(m + 1) * spc],
                        in_=rt_ps[:, m * spc:(m + 1) * spc],
                        func=AF.Identity,
                        bias=blin[:, m:m + 1],
                    )
                r_ps = tps.tile([spc, D], F32, tag="rps")
                for m in range(4):
                    nc.tensor.transpose(
                        out=r_ps[:, m * 128:(m + 1) * 128],
                        in_=rt_sb[:, m * spc:(m + 1) * spc],
                        identity=ident32[:],
                    )
                r_sb = tsb.tile([spc, D], F32, tag="rsb")
                nc.scalar.copy(r_sb[:], r_ps[:])
                sq = tsb.tile([spc, D], F32, tag="sq")
                nrm2 = tsb.tile([spc, 1], F32, tag="nrm2")
                nc.scalar.activation(out=sq[:], in_=r_sb[:], func=AF.Square,
                                     accum_out=nrm2[:])
                nrm = tsb.tile([spc, 1], F32, tag="nrm")
                nc.scalar.activation(out=nrm[:], in_=nrm2[:], func=AF.Sqrt)
                nc.vector.tensor_scalar_max(nrm[:], nrm[:], 1e-5)
                rcp = tsb.tile([spc, 1], F32, tag="rcp")
                nc.vector.reciprocal(rcp[:], nrm[:])
                o_sb = tsb.tile([spc, D], F32, tag="osb")
                nc.vector.tensor_scalar_mul(o_sb[:], r_sb[:], rcp[:])
                nc.sync.dma_start(out=out_d[:], in_=o_sb[:])

    nc.compile()
    return nc


def _wrap_idx(idx_flat):
    """[W] int16 -> [128, W/16] wrapped (i -> [i%16, i//16]) + replicated x8."""
    w = idx_flat.shape[0]
    blk = idx_flat.reshape(w // 16, 16).T
    return np.tile(blk, (8, 1))


def prep_host(inputs):
    tok_all = np.asarray(inputs["ast_path"]).astype(np.int64)
    apl = np.asarray(inputs["ast_path_len"]).astype(np.int64)
    emb = np.asarray(inputs["emb"], dtype=np.float32)
    n_total = tok_all.shape[0]
    b_total = apl.shape[0]
    assert n_total % NCORES == 0
    nloc = n_total // NCORES
    assert np.all(apl == apl[0]) and apl[0] * b_total == n_total, \
        "kernel assumes uniform paths-per-sample"
    pps = int(apl[0])
    assert nloc % pps == 0
    spc = b_total // NCORES

    lens_all = (tok_all != 0).sum(1)

    # balance samples across cores: snake assignment by total live-steps
    tot_per_sample = lens_all.reshape(b_total, pps).sum(1)
    order_s = np.argsort(-tot_per_sample, kind="stable")
    core_samples = [[] for _ in range(NCORES)]
    for r, sidx in enumerate(order_s):
        rnd, pos = divmod(r, NCORES)
        c = pos if rnd % 2 == 0 else NCORES - 1 - pos
        core_samples[c].append(int(sidx))

    orders, lens_sorted, core_rows = [], [], []
    sched = np.zeros(T, np.int64)
    min_s = [10 ** 9] * T
    max_s = [0] * T
    for c in range(NCORES):
        rows = np.concatenate([np.arange(s0 * pps, (s0 + 1) * pps)
                               for s0 in core_samples[c]])
        core_rows.append(rows)
        lens_c = lens_all[rows]
        order = np.argsort(-lens_c, kind="stable")
        orders.append(order)
        ls = lens_c[order]
        lens_sorted.append(ls)
        for t in range(T):
            sv = int((ls > t).sum())
            sched[t] = max(sched[t], sv)
            min_s[t] = min(min_s[t], sv)
            max_s[t] = max(max_s[t], sv)
    sched = tuple(int(w) for w in sched)                      # exact MM widths
    sched_g = tuple(-(-w // 128) * 128 for w in sched)        # gather widths
    min_s = tuple(int(v) for v in min_s)
    max_s = tuple(int(v) for v in max_s)
    sw, poff = _flag_offsets(sched_g, nloc)
    flag_len = int(poff[-1])

    emb_bf = (emb * 4.0).astype(FP8_NP)
    emb_bf[0, :] = 0

    def pack_w(wmat, scale):
        wt = (wmat.T * scale).astype(ml_dtypes.bfloat16)
        return np.concatenate([wt[k * 128:(k + 1) * 128, :] for k in range(2)],
                              axis=1).copy()

    def pack_w8(wmat, scale):
        wt = (wmat.T * scale).astype(FP8_NP)  # [E, 4H]
        pk = np.zeros((128, 2, wt.shape[1]), FP8_NP)
        pk[:, 0, :] = wt[0::2, :]
        pk[:, 1, :] = wt[1::2, :]
        return pk.reshape(128, -1).copy()

    wihf_h = pack_w8(np.asarray(inputs["W_ih_f"], np.float32), 2.0)
    whhf_h = pack_w(np.asarray(inputs["W_hh_f"], np.float32), SCALE_S)
    wihb_h = pack_w8(np.asarray(inputs["W_ih_b"], np.float32), 2.0)
    whhb_h = pack_w(np.asarray(inputs["W_hh_b"], np.float32), SCALE_S)
    wlin = np.asarray(inputs["W_lin"], np.float32)
    wlt_h = np.concatenate(
        [wlin.T[k * 128:(k + 1) * 128, :] for k in range(4)], axis=1
    ).astype(np.float32).copy()
    b4f_h = np.asarray(inputs["b_f"], np.float32).reshape(8, 128).T.copy()
    b4b_h = np.asarray(inputs["b_b"], np.float32).reshape(8, 128).T.copy()
    blin_h = np.asarray(inputs["b_lin"], np.float32).reshape(4, 128).T.copy()

    in_maps = []
    metas = []
    for c in range(NCORES):
        tok_c = tok_all[core_rows[c]]
        order = orders[c]
        tok_s = tok_c[order]
        ls = lens_sorted[c]

        gidx_blocks = []
        for t in range(T):
            w = sched_g[t]
            if w == 0:
                continue
            gidx_blocks.append(_wrap_idx(tok_s[:w, t].astype(np.int16)))
        gidx_h = np.concatenate(gidx_blocks, axis=1).copy() if gidx_blocks \
            else np.zeros((128, 0), np.int16)

        pad_full = (tok_s == 0).T.astype(np.float32)  # [T, nloc]
        padf_h = np.zeros(flag_len, np.float32)
        for t in range(T):
            padf_h[int(poff[t]):int(poff[t]) + sw[t]] = pad_full[t, :sw[t]]
        padf_h = padf_h.astype(FP8_NP).reshape(1, flag_len).copy()

        seg = (order // pps).astype(np.int64)
        ind_h = np.zeros((nloc, spc), np.float32)
        ind_h[np.arange(nloc), seg] = 1.0 / pps
        nblk = nloc // 128
        ind_flat = np.concatenate(
            [ind_h[j * 128:(j + 1) * 128, :] for j in range(nblk)], axis=1
        ).astype(ml_dtypes.bfloat16).copy()

        in_maps.append({
            "emb": emb_bf,
            "gidx": gidx_h,
            "padflag": padf_h,
            "ind": ind_flat,
            "wihf": wihf_h, "whhf": whhf_h,
            "wihb": wihb_h, "whhb": whhb_h,
            "wlt": wlt_h, "b4f": b4f_h, "b4b": b4b_h, "blin": blin_h,
        })
        metas.append({"order": order, "samples": core_samples[c]})
    return in_maps, sched, sched_g, nloc, spc, min_s, max_s, metas


def kernel(**inputs) -> np.ndarray:
    (in_maps, sched, sched_g, nloc, spc,
     min_s, max_s, metas) = prep_host(inputs)
    key = (sched, sched_g, nloc, spc, min_s, max_s)
    if key not in _NC_CACHE:
        _NC_CACHE[key] = build_nc(sched, sched_g, nloc, spc, min_s, max_s)
    nc = _NC_CACHE[key]
    res = run_bass_kernel_spmd(nc, in_maps, core_ids=list(range(NCORES)))
    b_total = len(metas) * spc
    out = np.zeros((b_total, 512), np.float32)
    for c in range(NCORES):
        oc = np.asarray(res.results[c]["out"], np.float32)
        for i, s0 in enumerate(metas[c]["samples"]):
            out[s0] = oc[i]
    return out


# revision 36
# speedup vs baseline: 1.0247x; 1.0247x over previous
"""Trainium2 Bass kernel for nn_AstPathEncoder (bidirectional LSTM + segment-mean).

Strategy (8 NeuronCores, data-parallel over paths):
  - Each core takes 2048 paths = 16 whole samples; weights replicated.
  - Per core, paths are sorted by descending length; at LSTM step t only the
    first W[t] (live, padded to 128) paths are processed.  W[t] is the max
    over cores so all 8 cores share one SPMD graph; per-core variation flows
    through input arrays only (gather indices, pad flags, the segment
    indicator matrix).
  - Embedding rows are fetched with dma_gather(transpose=True), which lands
    x^T [E x tokens] tiles directly in SBUF (gather + transpose fused).
  - Gates are computed transposed: gates[4H, n] = W_ih @ x^T + W_hh @ h^T
    (+bias via ACT), accumulated in PSUM with bf16 matmuls.
  - Backward dir: padding tokens force the input gate to -BIG via a K=1
    matmul with a per-core pad-flag row, so dead lanes keep (h,c) = 0 with
    no masking (packed-sequence semantics for the reverse scan).
  - Forward dir: final h per path is captured at its death step with a
    predicated copy against a broadcast death mask; the mask is built on the
    fly as (pad[t+1] - pad[t]) via two K=1 matmuls (padding is monotone).
  - Pooling: h_n^T is PE-transposed, then one indicator matmul produces the
    per-sample means (indicator holds 1/len); linear layer + L2 norm on-chip.
"""

import numpy as np
import ml_dtypes

import concourse.bass as bass
import concourse.mybir as mybir
import concourse.tile as tile
from concourse import bacc
from concourse.bass_utils import run_bass_kernel_spmd

F32 = mybir.dt.float32
BF16 = mybir.dt.bfloat16
FP8 = mybir.dt.float8e4
I16 = mybir.dt.int16
AF = mybir.ActivationFunctionType
OP = mybir.AluOpType

N, T, V, E, H, D, B = 16384, 16, 10000, 256, 256, 512, 128
NCORES = 8
BIG = 30.0
SCALE_S = 8.0   # fp8 x-path scale: emb*4, W_ih*2, W_hh*8, ACT scale 1/8

_NC_CACHE = {}

CDT = BF16  # cell-state dtype knob (BF16 or F32)

FP8_NP = ml_dtypes.float8_e4m3fn


def _flag_offsets(sched_g, nloc):
    """Stored width for pad row t is Wg[t-1] (Wg[-1] := nloc)."""
    sw = [nloc] + list(sched_g[:-1])
    poff = np.concatenate([[0], np.cumsum(sw)]).astype(int)
    return sw, poff


def build_nc(sched, sched_g, nloc, spc, min_s, max_s):
    nblk = nloc // 128
    gcols_per = [w // 16 for w in sched_g]
    goff = np.concatenate([[0], np.cumsum(gcols_per)]).astype(int)
    gcols = int(goff[-1])
    _, poff = _flag_offsets(sched_g, nloc)
    flag_len = int(poff[-1])

    nc = bacc.Bacc("TRN2", num_devices=NCORES)

    emb_d = nc.dram_tensor("emb", [V, E], FP8, kind="ExternalInput")
    gidx_d = nc.dram_tensor("gidx", [128, gcols], I16, kind="ExternalInput")
    pad_d = nc.dram_tensor("padflag", [1, flag_len], FP8, kind="ExternalInput")
    ind_d = nc.dram_tensor("ind", [128, nblk * spc], BF16, kind="ExternalInput")
    wihf_d = nc.dram_tensor("wihf", [128, 2 * 1024], FP8, kind="ExternalInput")
    whhf_d = nc.dram_tensor("whhf", [128, 2 * 1024], BF16, kind="ExternalInput")
    wihb_d = nc.dram_tensor("wihb", [128, 2 * 1024], FP8, kind="ExternalInput")
    whhb_d = nc.dram_tensor("whhb", [128, 2 * 1024], BF16, kind="ExternalInput")
    wlt_d = nc.dram_tensor("wlt", [128, 4 * D], F32, kind="ExternalInput")
    b4f_d = nc.dram_tensor("b4f", [128, 8], F32, kind="ExternalInput")
    b4b_d = nc.dram_tensor("b4b", [128, 8], F32, kind="ExternalInput")
    blin_d = nc.dram_tensor("blin", [128, 4], F32, kind="ExternalInput")
    out_d = nc.dram_tensor("out", [spc, D], F32, kind="ExternalOutput")

    with tile.TileContext(nc) as tc:
        with tc.tile_pool(name="persist", bufs=1) as pp:
            wihf = pp.tile([128, 2, 1024], FP8, tag="wihf")
            whhf = pp.tile([128, 2 * 1024], BF16, tag="whhf")
            wihb = pp.tile([128, 2, 1024], FP8, tag="wihb")
            whhb = pp.tile([128, 2 * 1024], BF16, tag="whhb")
            wlt = pp.tile([128, 4 * D], F32, tag="wlt")
            b4f = pp.tile([128, 8], F32, tag="b4f")
            b4b = pp.tile([128, 8], F32, tag="b4b")
            blin = pp.tile([128, 4], F32, tag="blin")
            gidx = pp.tile([128, gcols], I16, tag="gidx")
            padf = pp.tile([1, flag_len], FP8, tag="padf")
            ones_row = pp.tile([1, nloc], FP8, tag="ones_row")
            ind = pp.tile([128, nblk * spc], BF16, tag="ind")
            ones_c = pp.tile([1, 128], FP8, tag="ones_c")
            negones_c = pp.tile([1, 128], FP8, tag="negones_c")
            bigneg = pp.tile([1, 128], FP8, tag="bigneg")
            ident = pp.tile([128, 128], BF16, tag="ident")
            ident32 = pp.tile([128, 128], F32, tag="ident32")
            ident8 = pp.tile([128, 128], FP8, tag="ident8")
            # state (single-buffered h per dir: WAR deps follow the natural
            # per-step order, so no ping-pong needed)
            h_f = pp.tile([128, 2, nloc], BF16, tag="h_f")
            h_b = pp.tile([128, 2, nloc], BF16, tag="h_b")
            c_f = pp.tile([128, 2, nloc], CDT, tag="c_f")
            c_b = pp.tile([128, 2, nloc], CDT, tag="c_b")
            hf_fin = pp.tile([128, 2, nloc], BF16, tag="hffin")

            # gidx goes through the gpsimd queue so the first gather
            # follows it in-order with no cross-engine DMA-batch wait
            nc.gpsimd.dma_start(out=gidx[:], in_=gidx_d[:])
            nc.sync.dma_start(
                out=wihf[:], in_=wihf_d[:].rearrange("p (j m) -> p j m", j=2))
            nc.sync.dma_start(
                out=wihb[:], in_=wihb_d[:].rearrange("p (j m) -> p j m", j=2))
            for dst, src in ((padf, pad_d),
                             (whhf, whhf_d), (whhb, whhb_d),
                             (b4f, b4f_d), (b4b, b4b_d)):
                nc.sync.dma_start(out=dst[:], in_=src[:])
            nc.gpsimd.memset(ones_c[:], 1.0)
            nc.gpsimd.memset(negones_c[:], -1.0)
            nc.gpsimd.memset(bigneg[:], -BIG * SCALE_S)
            nc.gpsimd.memset(ones_row[:], 1.0)
            from concourse.masks import make_identity
            make_identity(nc, ident[:])
            make_identity(nc, ident32[:])
            make_identity(nc, ident8[:])

            def w_slice(wt, k, m):
                return wt[:, k * 1024 + m * 128:k * 1024 + (m + 1) * 128]

            def pad_sl(t, a, b):
                return padf[0:1, int(poff[t]) + a:int(poff[t]) + b]

            with tc.tile_pool(name="xt", bufs=4) as xp, \
                 tc.tile_pool(name="gates", bufs=2) as gp, \
                 tc.tile_pool(name="dvet", bufs=2) as vp, \
                 tc.tile_pool(name="psum", bufs=4, space="PSUM") as psp:

                xtiles = {}

                def emit_gather(dkey, t):
                    w = sched_g[t]
                    if w == 0:
                        return
                    xt = xp.tile([128, 2 * w], FP8, tag="xt", name="xt")
                    nc.gpsimd.dma_gather(
                        out_ap=xt[:].rearrange("p (c w) -> p c w", c=2),
                        in_ap=emb_d[:],
                        idxs_ap=gidx[:, int(goff[t]):int(goff[t + 1])],
                        num_idxs=w,
                        num_idxs_reg=w,
                        elem_size=E,
                        transpose=True,
                        single_packet=False,
                    )
                    xtiles[(dkey, t)] = xt

                def emit_step(t, fwd):
                    w = sched[t]
                    if w == 0:
                        return
                    first = (t == 0) if fwd else (t == T - 1)
                    wih = wihf if fwd else wihb
                    whh = whhf if fwd else whhb
                    b4 = b4f if fwd else b4b
                    h_st = h_f if fwd else h_b
                    cbuf = c_f if fwd else c_b
                    xt = xtiles.pop(("f" if fwd else "b", t))
                    gmap = ("i", "i", "f", "f", "g", "g", "o", "o")
                    gt = {}
                    for gname in ("i", "f", "g", "o"):
                        if first and gname == "f":
                            continue
                        gt[gname] = gp.tile([128, 2, w], BF16,
                                            tag="g" + gname, name="g" + gname)
                    death_lo = min_s[t + 1] if t < T - 1 else 0
                    death_hi = max_s[t]
                    has_death = fwd and (w > death_lo) and (death_hi > 0)
                    mk8 = None
                    if has_death:
                        mk8 = vp.tile([128, w], mybir.dt.uint8,
                                      tag="mk8", name="mk8", bufs=1)
                    for w0 in range(0, w, 1024):
                        wh = min(1024, w - w0)
                        # sigmoids (i, f, o) first, tanh (g) last per half
                        for m in (0, 1, 2, 3, 6, 7, 4, 5):
                            gname = gmap[m]
                            if first and gname == "f":
                                continue
                            ps = psp.tile([128, 1024], F32, tag="gp", name="gp")
                            n_k = 1 if first else 3
                            extra = (1 if (not fwd and m < 2
                                           and w0 + wh > min_s[t]) else 0)

                            def sub_extra(ws, sw):
                                return (extra and w0 + ws + sw > min_s[t])
                            ki = 0
                            for ws in range(0, wh, 512):
                                sw = min(512, wh - ws)
                                n_tot = n_k + (1 if sub_extra(ws, sw) else 0)
                                a = w0 + ws
                                rhs8 = xt[:, 2 * a:2 * (a + sw)].rearrange(
                                    "p (w j) -> p j w", j=2)
                                nc.tensor.matmul(
                                    out=ps[:, ws:ws + sw],
                                    lhsT=wih[:, :, m * 128:(m + 1) * 128],
                                    rhs=rhs8,
                                    start=True,
                                    stop=(ki == n_tot - 1),
                                    perf_mode=mybir.MatmulPerfMode.DoubleRow,
                                )
                            ki += 1
                            if not first:
                                for k in range(2):
                                    for ws in range(0, wh, 512):
                                        sw = min(512, wh - ws)
                                        n_tot = n_k + (1 if sub_extra(ws, sw) else 0)
                                        nc.tensor.matmul(
                                            out=ps[:, ws:ws + sw],
                                            lhsT=w_slice(whh, k, m),
                                            rhs=h_st[:, k, w0 + ws:w0 + ws + sw],
                                            start=False,
                                            stop=(ki == n_tot - 1),
                                        )
                                    ki += 1
                            if extra:
                                for ws in range(0, wh, 512):
                                    sw = min(512, wh - ws)
                                    if not sub_extra(ws, sw):
                                        continue
                                    nc.tensor.matmul(
                                        out=ps[:, ws:ws + sw],
                                        lhsT=bigneg[:],
                                        rhs=pad_sl(t, w0 + ws, w0 + ws + sw),
                                        start=False,
                                        stop=True,
                                    )
                            func = AF.Tanh if gname == "g" else AF.Sigmoid
                            nc.scalar.activation(
                                out=gt[gname][:, m % 2, w0:w0 + wh],
                                in_=ps[:, :wh],
                                func=func,
                                bias=b4[:, m:m + 1],
                                scale=1.0 / SCALE_S,
                            )
                        if has_death and (w0 + wh > death_lo):
                            mask_ps = psp.tile([128, 1024], F32, tag="gp",
                                               name="mask_ps")
                            for ws in range(0, wh, 512):
                                sw = min(512, wh - ws)
                                a, bnd = w0 + ws, w0 + ws + sw
                                rhs_next = (pad_sl(t + 1, a, bnd) if t < T - 1
                                            else ones_row[0:1, a:bnd])
                                nc.tensor.matmul(
                                    out=mask_ps[:, ws:ws + sw], lhsT=ones_c[:],
                                    rhs=rhs_next, start=True, stop=False)
                                nc.tensor.matmul(
                                    out=mask_ps[:, ws:ws + sw], lhsT=negones_c[:],
                                    rhs=pad_sl(t, a, bnd),
                                    start=False, stop=True)
                            nc.vector.tensor_copy(mk8[:, w0:w0 + wh],
                                                  mask_ps[:, :wh])
                        # cell update on DVE for this half
                        c_sl = cbuf[:, :, w0:w0 + wh]
                        if first:
                            nc.vector.tensor_tensor(
                                out=c_sl, in0=gt["i"][:, :, w0:w0 + wh],
                                in1=gt["g"][:, :, w0:w0 + wh], op=OP.mult)
                        else:
                            ig = vp.tile([128, 2, wh], BF16, tag="ig",
                                         name="ig", bufs=1)
                            nc.vector.tensor_tensor(
                                out=ig[:], in0=gt["i"][:, :, w0:w0 + wh],
                                in1=gt["g"][:, :, w0:w0 + wh], op=OP.mult)
                            nc.vector.tensor_tensor(
                                out=c_sl, in0=gt["f"][:, :, w0:w0 + wh],
                                in1=c_sl, op=OP.mult)
                            nc.vector.tensor_tensor(
                                out=c_sl, in0=c_sl, in1=ig[:], op=OP.add)
                        tc_t = vp.tile([128, 2, wh], BF16, tag="tc", name="tc")
                        nc.scalar.activation(out=tc_t[:], in_=c_sl, func=AF.Tanh)
                        h_sl = h_st[:, :, w0:w0 + wh]
                        nc.vector.tensor_tensor(
                            out=h_sl, in0=gt["o"][:, :, w0:w0 + wh],
                            in1=tc_t[:], op=OP.mult)
                        if mk8 is not None and (w0 + wh > death_lo):
                            for ch in range(2):
                                nc.vector.copy_predicated(
                                    out=hf_fin[:, ch, w0:w0 + wh],
                                    mask=mk8[:, w0:w0 + wh],
                                    data=h_st[:, ch, w0:w0 + wh],
                                )

                emit_gather("b", T - 1)
                emit_gather("f", 0)
                nc.vector.memset(h_b[:], 0.0)
                nc.vector.memset(c_b[:], 0.0)
                for i in range(T):
                    if i + 1 < T:
                        emit_gather("f", i + 1)
                        emit_gather("b", T - 2 - i)
                    emit_step(i, fwd=True)
                    emit_step(T - 1 - i, fwd=False)

            for dst, src in ((wlt, wlt_d), (blin, blin_d), (ind, ind_d)):
                nc.sync.dma_start(out=dst[:], in_=src[:])
            # ---------------- tail: pooling + linear + L2 norm ----------------
            with tc.tile_pool(name="tailsb", bufs=2) as tsb, \
                 tc.tile_pool(name="tailps", bufs=1, space="PSUM") as tps, \
                 tc.tile_pool(name="tailps2", bufs=2, space="PSUM") as tps2:
                pool_ps = tps.tile([spc, D], F32, tag="pool16")
                for j in range(nblk):
                    tp = tps2.tile([128, 512], BF16, tag="tp", name="tp")
                    for q in range(4):
                        src = (hf_fin if q < 2 else h_b)
                        ch = q % 2
                        nc.tensor.transpose(
                            out=tp[:, q * 128:(q + 1) * 128],
                            in_=src[:, ch, j * 128:(j + 1) * 128],
                            identity=ident[:],
                        )
                    hnt = tsb.tile([128, 512], BF16, tag="hnt", name="hnt")
                    nc.scalar.copy(hnt[:], tp[:])
                    nc.tensor.matmul(
                        out=pool_ps[:],
                        lhsT=ind[:, j * spc:(j + 1) * spc],
                        rhs=hnt[:],
                        start=(j == 0),
                        stop=(j == nblk - 1),
                    )
                pool_sb = tsb.tile([spc, D], F32, tag="poolsb")
                nc.scalar.copy(pool_sb[:], pool_ps[:])
                pt_ps = tps.tile([128, 4 * spc], F32, tag="ptps")
                for q in range(4):
                    nc.tensor.transpose(
                        out=pt_ps[:, q * spc:(q + 1) * spc],
                        in_=pool_sb[:, q * 128:(q + 1) * 128],
                        identity=ident32[:spc, :spc],
                    )
                pt_sb = tsb.tile([128, 4 * spc], F32, tag="ptsb")
                nc.scalar.copy(pt_sb[:], pt_ps[:])
                rt_ps = tps.tile([128, 4 * spc], F32, tag="rtps")
                for m in range(4):
                    for k in range(4):
                        nc.tensor.matmul(
                            out=rt_ps[:, m * spc:(m + 1) * spc],
                            lhsT=wlt[:, k * D + m * 128:k * D + (m + 1) * 128],
                            rhs=pt_sb[:, k * spc:(k + 1) * spc],
                            start=(k == 0),
                            stop=(k == 3),
                        )
                rt_sb = tsb.tile([128, 4 * spc], F32, tag="rtsb")
                for m in range(4):
                    nc.scalar.activation(
                        out=rt_sb[:, m * spc:(m + 1) * spc],
                        in_=rt_ps[:, m * spc:(m + 1) * spc],
                        func=AF.Identity,
                        bias=blin[:, m:m + 1],
                    )
                r_ps = tps.tile([spc, D], F32, tag="rps")
                for m in range(4):
                    nc.tensor.transpose(
                        out=r_ps[:, m * 128:(m + 1) * 128],
                        in_=rt_sb[:, m * spc:(m + 1) * spc],
                        identity=ident32[:],
                    )
                r_sb = tsb.tile([spc, D], F32, tag="rsb")
                nc.scalar.copy(r_sb[:], r_ps[:])
                sq = tsb.tile([spc, D], F32, tag="sq")
                nrm2 = tsb.tile([spc, 1], F32, tag="nrm2")
                nc.scalar.activation(out=sq[:], in_=r_sb[:], func=AF.Square,
                                     accum_out=nrm2[:])
                nrm = tsb.tile([spc, 1], F32, tag="nrm")
                nc.scalar.activation(out=nrm[:], in_=nrm2[:], func=AF.Sqrt)
                nc.vector.tensor_scalar_max(nrm[:], nrm[:], 1e-5)
                rcp = tsb.tile([spc, 1], F32, tag="rcp")
                nc.vector.reciprocal(rcp[:], nrm[:])
                o_sb = tsb.tile([spc, D], F32, tag="osb")
                nc.vector.tensor_scalar_mul(o_sb[:], r_sb[:], rcp[:])
                nc.sync.dma_start(out=out_d[:], in_=o_sb[:])

    nc.compile()
    return nc


def _wrap_idx(idx_flat):
    """[W] int16 -> [128, W/16] wrapped (i -> [i%16, i//16]) + replicated x8."""
    w = idx_flat.shape[0]
    blk = idx_flat.reshape(w // 16, 16).T
    return np.tile(blk, (8, 1))


def prep_host(inputs):
    tok_all = np.asarray(inputs["ast_path"]).astype(np.int64)
    apl = np.asarray(inputs["ast_path_len"]).astype(np.int64)
    emb = np.asarray(inputs["emb"], dtype=np.float32)
    n_total = tok_all.shape[0]
    b_total = apl.shape[0]
    assert n_total % NCORES == 0
    nloc = n_total // NCORES
    assert np.all(apl == apl[0]) and apl[0] * b_total == n_total, \
        "kernel assumes uniform paths-per-sample"
    pps = int(apl[0])
    assert nloc % pps == 0
    spc = b_total // NCORES

    lens_all = (tok_all != 0).sum(1)

    # balance samples across cores: snake assignment by total live-steps
    tot_per_sample = lens_all.reshape(b_total, pps).sum(1)
    order_s = np.argsort(-tot_per_sample, kind="stable")
    core_samples = [[] for _ in range(NCORES)]
    for r, sidx in enumerate(order_s):
        rnd, pos = divmod(r, NCORES)
        c = pos if rnd % 2 == 0 else NCORES - 1 - pos
        core_samples[c].append(int(sidx))

    orders, lens_sorted, core_rows = [], [], []
    sched = np.zeros(T, np.int64)
    min_s = [10 ** 9] * T
    max_s = [0] * T
    for c in range(NCORES):
        rows = np.concatenate([np.arange(s0 * pps, (s0 + 1) * pps)
                               for s0 in core_samples[c]])
        core_rows.append(rows)
        lens_c = lens_all[rows]
        order = np.argsort(-lens_c, kind="stable")
        orders.append(order)
        ls = lens_c[order]
        lens_sorted.append(ls)
        for t in range(T):
            sv = int((ls > t).sum())
            sched[t] = max(sched[t], sv)
            min_s[t] = min(min_s[t], sv)
            max_s[t] = max(max_s[t], sv)
    sched = tuple(int(w) for w in sched)                      # exact MM widths
    sched_g = tuple(-(-w // 128) * 128 for w in sched)        # gather widths
    min_s = tuple(int(v) for v in min_s)
    max_s = tuple(int(v) for v in max_s)
    sw, poff = _flag_offsets(sched_g, nloc)
    flag_len = int(poff[-1])

    emb_bf = (emb * 4.0).astype(FP8_NP)
    emb_bf[0, :] = 0

    def pack_w(wmat, scale):
        wt = (wmat.T * scale).astype(ml_dtypes.bfloat16)
        return np.concatenate([wt[k * 128:(k + 1) * 128, :] for k in range(2)],
                              axis=1).copy()

    def pack_w8(wmat, scale):
        wt = (wmat.T * scale).astype(FP8_NP)  # [E, 4H]
        pk = np.zeros((128, 2, wt.shape[1]), FP8_NP)
        pk[:, 0, :] = wt[0::2, :]
        pk[:, 1, :] = wt[1::2, :]
        return pk.reshape(128, -1).copy()

    wihf_h = pack_w8(np.asarray(inputs["W_ih_f"], np.float32), 2.0)
    whhf_h = pack_w(np.asarray(inputs["W_hh_f"], np.float32), SCALE_S)
    wihb_h = pack_w8(np.asarray(inputs["W_ih_b"], np.float32), 2.0)
    whhb_h = pack_w(np.asarray(inputs["W_hh_b"], np.float32), SCALE_S)
    wlin = np.asarray(inputs["W_lin"], np.float32)
    wlt_h = np.concatenate(
        [wlin.T[k * 128:(k + 1) * 128, :] for k in range(4)], axis=1
    ).astype(np.float32).copy()
    b4f_h = np.asarray(inputs["b_f"], np.float32).reshape(8, 128).T.copy()
    b4b_h = np.asarray(inputs["b_b"], np.float32).reshape(8, 128).T.copy()
    blin_h = np.asarray(inputs["b_lin"], np.float32).reshape(4, 128).T.copy()

    in_maps = []
    metas = []
    for c in range(NCORES):
        tok_c = tok_all[core_rows[c]]
        order = orders[c]
        tok_s = tok_c[order]
        ls = lens_sorted[c]

        gidx_blocks = []
        for t in range(T):
            w = sched_g[t]
            if w == 0:
                continue
            gidx_blocks.append(_wrap_idx(tok_s[:w, t].astype(np.int16)))
        gidx_h = np.concatenate(gidx_blocks, axis=1).copy() if gidx_blocks \
            else np.zeros((128, 0), np.int16)

        pad_full = (tok_s == 0).T.astype(np.float32)  # [T, nloc]
        padf_h = np.zeros(flag_len, np.float32)
        for t in range(T):
            padf_h[int(poff[t]):int(poff[t]) + sw[t]] = pad_full[t, :sw[t]]
        padf_h = padf_h.astype(FP8_NP).reshape(1, flag_len).copy()

        seg = (order // pps).astype(np.int64)
        ind_h = np.zeros((nloc, spc), np.float32)
        ind_h[np.arange(nloc), seg] = 1.0 / pps
        nblk = nloc // 128
        ind_flat = np.concatenate(
            [ind_h[j * 128:(j + 1) * 128, :] for j in range(nblk)], axis=1
        ).astype(ml_dtypes.bfloat16).copy()

        in_maps.append({
            "emb": emb_bf,
            "gidx": gidx_h,
            "padflag": padf_h,
            "ind": ind_flat,
            "wihf": wihf_h, "whhf": whhf_h,
            "wihb": wihb_h, "whhb": whhb_h,
            "wlt": wlt_h, "b4f": b4f_h, "b4b": b4b_h, "blin": blin_h,
        })
        metas.append({"order": order, "samples": core_samples[c]})
    return in_maps, sched, sched_g, nloc, spc, min_s, max_s, metas


def kernel(**inputs) -> np.ndarray:
    (in_maps, sched, sched_g, nloc, spc,
     min_s, max_s, metas) = prep_host(inputs)
    key = (sched, sched_g, nloc, spc, min_s, max_s)
    if key not in _NC_CACHE:
        _NC_CACHE[key] = build_nc(sched, sched_g, nloc, spc, min_s, max_s)
    nc = _NC_CACHE[key]
    res = run_bass_kernel_spmd(nc, in_maps, core_ids=list(range(NCORES)))
    b_total = len(metas) * spc
    out = np.zeros((b_total, 512), np.float32)
    for c in range(NCORES):
        oc = np.asarray(res.results[c]["out"], np.float32)
        for i, s0 in enumerate(metas[c]["samples"]):
            out[s0] = oc[i]
    return out
